# revision 5
# baseline (speedup 1.0000x reference)
"""Distributed multi-head attention kernel for 8 Trainium2 NeuronCores (v2).

Problem: y = softmax((x Wq^T)(x Wk^T)^T / sqrt(D)) (x Wv^T) Wo^T + bo
with B=4, T=2048, C=1280, H=20, D=64, float32 I/O.

Sharding (Megatron-style batch x head tensor parallel):
  Core r owns batch b = r//2 and heads [10*(r%2), 10*(r%2)+10).  Each
  core projects Q/K/V for its 10 heads from the full batch-b token
  stream, runs attention for all 2048 queries over its heads, applies
  its half of the output projection (row-split Wo), and exchanges the
  partial outputs with its pair core via PER-CHUNK pairwise
  ReduceScatters so each core ends up with the final activations for
  the first (even rank) / second (odd rank) 256 tokens of every
  512-token chunk.

v2 changes vs the baseline:
  - the two big ReduceScatters (one mid-kernel, one in the tail) are
    replaced by 8 small per-chunk-half ReduceScatters that are emitted
    lazily from the filler queue right after the outproj DMAs they
    consume, so the gpsimd queue (otherwise idle) is the only engine
    that ever waits on a collective;
  - the RS readback DMA runs on the gpsimd queue too (the baseline put
    a collective-gated DMA on the Sync engine, head-of-line blocking
    every later DMA for ~43us mid-kernel);
  - softmax normalization is per-head-pair inside the chunk (recip +
    broadcast queued right after the P@V staging), so a chunk's
    normalized attention output is complete at chunk end and its
    output projection can run as filler in the next chunk;
  - final bias+store for chunk qc runs at the start of chunk qc+2
    (vector + Sync DMA, data long since ready);
  - the ramp starts attention after only K(hp0, chunk0)+Q(hp0, qc0)
    using small per-slice weight DMAs; the remaining K chunks are
    drained just-in-time inside the first kt loop.

Attention core (unchanged): transposed "S_T[k, q]" layout, V padded
per head to 65 with a ones column so psum row 64 of P@V is the softmax
denominator; exp() on ScalarE out of PSUM in [128, 1024] chunks;
filler projections dribbled into the PE stream between S_T matmuls.
Compute dtype bf16, fp32 PSUM accumulation, fp32 I/O.
"""

import sys
import types
from collections import deque

import numpy as np
import ml_dtypes

import concourse.bass as bass
import concourse.mybir as mybir
import concourse.tile as tile
from concourse import bacc
from concourse.bass_utils import run_bass_kernel_spmd

N_CORES = 8
C = 1280          # model width
CL = 640          # local width (10 heads)
HL = 10           # local heads
HP = 5            # local head-pair tiles (128 rows = 2 heads x 64)
D = 64
B = 4
T = 2048          # full sequence length
TQ = 512          # query chunk
QC = T // TQ      # 4 query chunks
KT = T // 128     # 16 key tiles
CI = C // 128     # 10 contraction tiles
CO = C // 128     # 10 output tiles
CLT = CL // 128   # 5 local-channel tiles
PSL = 7           # P slab ring depth (key tiles resident)
BF = mybir.dt.bfloat16
F32 = mybir.dt.float32
F8 = mybir.dt.float8e4
VSC = 16.0        # host pre-scale on Wv to keep fp8 out of subnormals
SCALE = 1.0 / (D ** 0.5)

LAST_EXEC_TIME_NS = None
_BUILD_CACHE = {}


def _install_ntff_hook():
    """The trimmed antenv package lacks axon_hooks; register the NTFF
    profile hook by hand so trace=True can time the NEFF on silicon.
    Safe no-op if anything is missing."""
    if "antenv.axon_hooks" in sys.modules:
        return
    try:
        from trn_agent_boot.trn_boot import _ntff_profile_via_ctypes

        hook = _ntff_profile_via_ctypes("/opt/axon/libaxon_pjrt.so")
        mod = types.ModuleType("antenv.axon_hooks")
        mod.get_axon_ntff_profile_hook = lambda: hook
        mod.set_axon_ntff_profile_hook = lambda h: None
        sys.modules["antenv.axon_hooks"] = mod
        import antenv

        antenv.axon_hooks = mod
    except Exception:
        pass


def build():
    RG2 = [[2 * i, 2 * i + 1] for i in range(N_CORES // 2)]

    nc = bacc.Bacc("TRN2", target_bir_lowering=False, debug=False,
                   num_devices=N_CORES)

    xT = nc.dram_tensor("xT", [C, T], BF, kind="ExternalInput").ap()
    wqT = nc.dram_tensor("wqT", [C, CL], BF, kind="ExternalInput").ap()
    wkT = nc.dram_tensor("wkT", [C, CL], BF, kind="ExternalInput").ap()
    wvT = nc.dram_tensor("wvT", [C, CL], BF, kind="ExternalInput").ap()
    woT = nc.dram_tensor("woT", [CL, C], BF, kind="ExternalInput").ap()
    bo_d = nc.dram_tensor("bo", [C, 1], F32, kind="ExternalInput").ap()
    sel_d = nc.dram_tensor("sel", [HL, HP * 128], BF,
                           kind="ExternalInput").ap()
    # out[:, qc*256:(qc+1)*256] = final activations for tokens
    # qc*512 + rank%2 * 256 + [0, 256) of this core's batch
    out = nc.dram_tensor("out", [C, QC * 256], F32, kind="ExternalOutput").ap()

    with tile.TileContext(nc) as tc:
        with tc.tile_pool(name="dram", bufs=1, space="DRAM") as dram:
            # per-chunk-half ReduceScatter bounce buffers:
            # rs_in[qc][half] = [slot s(2), co(5), 128, 256]; slot 0 =
            # token cols 0:256 (kept by even rank), slot 1 = 256:512.
            HSZ = 5 * 128 * 256
            rs_in = [[dram.tile([2 * HSZ], BF, name=f"rsin{q}_{h}")
                      for h in range(2)] for q in range(QC)]
            y_red = [[dram.tile([HSZ], BF, name=f"yred{q}_{h}")
                      for h in range(2)] for q in range(QC)]
            rs_in_v = [[t[:].rearrange("(s o p t) -> s o p t", s=2, o=5,
                                       p=128)
                        for t in row] for row in rs_in]
            y_red_v = [[t[:].rearrange("(o p t) -> p o t", o=5, p=128)
                        for t in row] for row in y_red]

            with tc.tile_pool(name="sb", bufs=1) as sb, \
                 tc.tile_pool(name="psum", bufs=1, space="PSUM") as psum:
                xT_sb = sb.tile([128, CI, T], BF)
                wq_sb = sb.tile([128, CI, CL], BF)
                wv_sb = sb.tile([128, CI, CL], BF)
                wo_sb = sb.tile([128, CLT, C], BF)
                bo_sb = sb.tile([128, CO, 1], F32)
                kT_sb = sb.tile([128, HP, T], BF)
                # only the current chunk's queries are live: buffer by
                # chunk parity instead of holding all T
                qT_sb = sb.tile([128, HP, 2 * TQ], BF)
                # token-major V, per-head 65-padded with a ones column
                vb = sb.tile([128, KT, HL * 65], BF)
                # staging for attention outputs of one qc (double buffered)
                attn_sb = [sb.tile([128, CLT, TQ], BF, name=f"attn{i}")
                           for i in range(2)]
                # RS result readback (double buffered by chunk parity)
                rb = [sb.tile([128, CO, 256], BF, name=f"rb{i}")
                      for i in range(2)]
                # staged P@V results ([64 dims | row 64 = denominator])
                pav_sb = {}
                for hp in range(HP):
                    for h in range(2):
                        pav_sb[(hp, h)] = sb.tile([65, TQ], BF,
                                                  name=f"pav_sb{hp}_{h}")
                den_sb = [sb.tile([HL, TQ], BF, name=f"den{i}")
                          for i in range(2)]
                den_f = sb.tile([HL, TQ], F32, name="den_f")
                rec_f = sb.tile([HL, TQ], F32, name="rec_f")
                rec_sb = [sb.tile([HL, TQ], BF, name=f"rec{i}")
                          for i in range(2)]
                # selector for broadcasting recip rows across partitions:
                # sel[p, hp, h, i] = 1 iff p == 2*hp + h
                sel_sb = sb.tile([HL, HP, 2, 64], BF, name="sel")
                # DMA priority order: the ramp needs sel, x chunk 0, the
                # K(hp0)/Q(hp0) weight slices and the V(hp0) slice first.
                nc.sync.dma_start(
                    sel_sb[:],
                    sel_d.rearrange("p (n h o) -> p n h o", h=2, o=64))
                # per-hp recip recomputes all rows; rows later hps have
                # not produced yet must be finite
                nc.vector.memset(den_sb[0][:], 1.0)
                nc.vector.memset(den_sb[1][:], 1.0)
                xT_v = xT.rearrange("(n p) t -> p n t", p=128)
                # chunk 0 lands per contraction tile so the ramp's first
                # K/Q matmuls start on tile i as soon as it arrives
                # instead of waiting for the whole 1.3MB transfer
                for ci in range(CI):
                    nc.sync.dma_start(xT_sb[:, ci, 0:TQ],
                                      xT_v[:, ci, 0:TQ])
                wq_v = wqT.rearrange("(n p) o -> p n o", p=128)
                wk_v = wkT.rearrange("(n p) o -> p n o", p=128)

                vb_h = vb[:].rearrange("p k (h c) -> p k h c", c=65)
                nc.vector.memset(vb_h[:, :, :, 64:65], 1.0)

                with tc.tile_pool(name="p2", bufs=1) as p2:
                    # exp(S_T) ring: PSL key tiles of [headA 512|headB 512]
                    Pslab = p2.tile([128, PSL, 1024], BF)

                    def ps_tile(name):
                        return psum.tile([128, 1024], F32, tag="ps",
                                         bufs=2, name=name)

                    def fill_tile(name):
                        return psum.tile([128, TQ], F32, tag="fill",
                                         bufs=2, name=name)

                    def wk_slice(hp):
                        wk_sl = p2.tile([128, CI, 128], BF, tag="wk",
                                        bufs=2, name="wk_sl")
                        nc.sync.dma_start(
                            wk_sl[:], wk_v[:, :, hp * 128:(hp + 1) * 128])
                        return wk_sl

                    # ---- filler machinery: FIFO of (label, generator)
                    # groups emitting one PE matmul per step
                    queue = deque()

                    def step_filler(n):
                        done = 0
                        while done < n and queue:
                            try:
                                next(queue[0][1])
                                done += 1
                            except StopIteration:
                                queue.popleft()

                    def drain_label(label):
                        while any(g[0] == label for g in queue):
                            step_filler(16)

                    def drain_filler():
                        while queue:
                            step_filler(64)

                    def g_proj(w_sb, wi, dst_sb, ot, t0, d0=None):
                        # dst[o, d0+t] = sum_i W^T[i, o] x^T[i, t0+t]
                        if d0 is None:
                            d0 = t0
                        ps = fill_tile("ps_proj")
                        for i in range(CI):
                            nc.tensor.matmul(
                                ps[:],
                                w_sb[:, i, wi * 128:(wi + 1) * 128],
                                xT_sb[:, i, t0:t0 + TQ],
                                start=(i == 0), stop=(i == CI - 1))
                            yield
                        nc.vector.tensor_copy(dst_sb[:, ot, d0:d0 + TQ],
                                              ps[:])

                    def g_out_proj(pb, co, qc):
                        # partial y[co, tokens] = Wo_loc^T[:, co] @ attn;
                        # halves go to the RS slots (token cols 0:256 ->
                        # slot 0 kept by even rank, 256:512 -> slot 1)
                        psy = fill_tile("psy")
                        for ct in range(CLT):
                            nc.tensor.matmul(
                                psy[:],
                                wo_sb[:, ct, co * 128:(co + 1) * 128],
                                attn_sb[pb][:, ct, :],
                                start=(ct == 0), stop=(ct == CLT - 1))
                            yield
                        ysb = p2.tile([128, TQ], BF, tag="ysb", bufs=2,
                                      name="ysb")
                        nc.vector.tensor_copy(ysb[:], psy[:])
                        v = rs_in_v[qc][co // 5]
                        nc.sync.dma_start(v[0, co % 5], ysb[:, 0:256])
                        nc.sync.dma_start(v[1, co % 5], ysb[:, 256:512])

                    def gen_rs(qc, half):
                        # lazily emitted from the filler queue AFTER the
                        # 5 outproj groups of this half, so the collective
                        # waits on all its input DMAs (tile deps are
                        # emission-ordered).  Runs on the gpsimd queue,
                        # which nothing latency-critical shares.
                        nc.gpsimd.collective_compute(
                            "ReduceScatter", mybir.AluOpType.add,
                            replica_groups=RG2,
                            ins=[rs_in[qc][half][:].opt()],
                            outs=[y_red[qc][half][:].opt()])
                        nc.gpsimd.dma_start(
                            rb[qc % 2][:, half * 5:half * 5 + 5, :],
                            y_red_v[qc][half])
                        yield

                    def g_norm(pb, hp, h):
                        # broadcast 1/den over 64 partitions and normalize;
                        # head 0 lands on partitions 0:64 so the multiply
                        # can write attn_sb directly (no bounce DMA)
                        pbc = fill_tile("pbc")
                        nc.tensor.matmul(
                            pbc[0:64, :],
                            sel_sb[:, hp, h, :],
                            rec_sb[pb][:],
                            start=True, stop=True)
                        yield
                        if h == 0:
                            nc.vector.tensor_mul(
                                attn_sb[pb][0:64, hp, :],
                                pav_sb[(hp, 0)][0:64, :], pbc[0:64, :])
                        else:
                            tmp = p2.tile([64, TQ], BF, tag="tmp", bufs=2,
                                          name="tmp")
                            nc.vector.tensor_mul(
                                tmp[:], pav_sb[(hp, 1)][0:64, :],
                                pbc[0:64, :])
                            nc.sync.dma_start(
                                attn_sb[pb][64:128, hp, :], tmp[:])

                    def recip_hp(pb):
                        nc.vector.tensor_copy(den_f[:], den_sb[pb][:])
                        nc.vector.reciprocal_approx_fast(rec_f[:], den_f[:])
                        nc.vector.tensor_copy(rec_sb[pb][:], rec_f[:])

                    def run_inline(gen):
                        for _ in gen:
                            pass

                    def proj_v(tt, hp):
                        # V for token tile tt, head pair hp (token-major)
                        ps = ps_tile("ps_v")
                        for i in range(CI):
                            nc.tensor.matmul(
                                ps[:, :128],
                                xT_sb[:, i, tt * 128:(tt + 1) * 128],
                                wv_sb[:, i, hp * 128:(hp + 1) * 128],
                                start=(i == 0), stop=(i == CI - 1))
                        nc.vector.tensor_copy(
                            vb_h[:, tt, 2 * hp:2 * hp + 2, 0:64],
                            ps[:, :128].rearrange("p (h c) -> p h c", c=64))

                    def final(qc):
                        # add bias to the reduced output and store; rb data
                        # has been ready since mid-previous chunk
                        for co in range(CO):
                            fo = p2.tile([128, 256], F32, tag="fo", bufs=4,
                                         name="fo")
                            nc.vector.tensor_scalar_add(
                                fo[:], rb[qc % 2][:, co, :], bo_sb[:, co, :])
                            nc.sync.dma_start(
                                out[co * 128:(co + 1) * 128,
                                    qc * 256:(qc + 1) * 256],
                                fo[:])

                    # ---- ramp: only K(hp0, chunk0) + Q(hp0, qc0) inline;
                    # the remaining K chunks drain just-in-time inside the
                    # first kt loop.  A throwaway exp warms the ACT table.
                    nc.scalar.activation(
                        rec_f[0:1, 0:1], den_f[0:1, 0:1],
                        mybir.ActivationFunctionType.Exp, scale=SCALE)
                    wk0 = wk_slice(0)
                    nc.sync.dma_start(wq_sb[:, :, 0:128], wq_v[:, :, 0:128])
                    wv_v = wvT.rearrange("(n p) o -> p n o", p=128)
                    nc.sync.dma_start(wv_sb[:, :, 0:128],
                                      wv_v[:, :, 0:128])
                    for tc4 in range(1, QC):
                        nc.sync.dma_start(
                            xT_sb[:, :, tc4 * TQ:(tc4 + 1) * TQ],
                            xT_v[:, :, tc4 * TQ:(tc4 + 1) * TQ])
                    nc.sync.dma_start(wq_sb[:, :, 128:CL],
                                      wq_v[:, :, 128:CL])
                    nc.sync.dma_start(wv_sb[:, :, 128:CL],
                                      wv_v[:, :, 128:CL])
                    nc.sync.dma_start(
                        wo_sb[:], woT.rearrange("(n p) o -> p n o", p=128))
                    nc.sync.dma_start(
                        bo_sb[:], bo_d.rearrange("(n p) o -> p n o", p=128))
                    run_inline(g_proj(wk0, 0, kT_sb, 0, 0))
                    run_inline(g_proj(wq_sb, 0, qT_sb, 0, 0))
                    for tc4 in range(1, QC):
                        queue.append((("k0", tc4), g_proj(
                            wk0, 0, kT_sb, 0, tc4 * TQ)))

                    for pidx in range(QC):
                        qc = pidx
                        pb = pidx % 2
                        first = pidx == 0
                        if first:
                            for hp in range(1, HP):
                                wk_sl = wk_slice(hp)
                                for tc4 in range(QC):
                                    queue.append((("k", hp), g_proj(
                                        wk_sl, 0, kT_sb, hp, tc4 * TQ)))
                                queue.append((("q", hp), g_proj(
                                    wq_sb, hp, qT_sb, hp, qc * TQ,
                                    (qc % 2) * TQ)))
                        else:
                            queue.append((("q", 1), g_proj(
                                wq_sb, 1, qT_sb, 1, qc * TQ,
                                (qc % 2) * TQ)))
                            for ot in range(2, HP):
                                queue.append((("q", ot), g_proj(
                                    wq_sb, ot, qT_sb, ot, qc * TQ,
                                    (qc % 2) * TQ)))
                            # previous chunk's output projection + lazy RS
                            for co in range(CO):
                                queue.append((("o", co),
                                              g_out_proj(1 - pb, co,
                                                         qc - 1)))
                                if co == 4:
                                    queue.append((("rs", qc - 1, 0),
                                                  gen_rs(qc - 1, 0)))
                            queue.append((("rs", qc - 1, 1),
                                          gen_rs(qc - 1, 1)))

                        for hp in range(HP):
                            if hp:
                                drain_label(("k", hp))
                                drain_label(("q", hp))
                            elif not first:
                                drain_label(("q", 0))
                            if hp == 2 and pidx >= 2:
                                # bias+store for chunk qc-2: emitted
                                # mid-chunk where the Vector FIFO has
                                # slack (at chunk start its WAR chain on
                                # the fo ring stalled staging for ~10us)
                                final(pidx - 2)
                            pavs = [psum.tile([65, TQ], F32, tag="pav",
                                              bufs=2, name=f"pav{h}")
                                    for h in (0, 1)]

                            def s_t(kt):
                                # S_T[k in tile kt, q] for both heads
                                ps = ps_tile("ps_st")
                                for h in (0, 1):
                                    nc.tensor.matmul(
                                        ps[:, h * TQ:(h + 1) * TQ],
                                        kT_sb[h * 64:(h + 1) * 64, hp,
                                              kt * 128:(kt + 1) * 128],
                                        qT_sb[h * 64:(h + 1) * 64, hp,
                                              (qc % 2) * TQ:
                                              (qc % 2 + 1) * TQ],
                                        start=True, stop=True,
                                        tile_position=(h * 64, 0))
                                nc.scalar.activation(
                                    Pslab[:, kt % PSL, :], ps[:],
                                    mybir.ActivationFunctionType.Exp,
                                    scale=SCALE)

                            def pv(kt):
                                for h in (0, 1):
                                    nc.tensor.matmul(
                                        pavs[h][:],
                                        vb_h[:, kt, 2 * hp + h, :],
                                        Pslab[:, kt % PSL,
                                              h * TQ:(h + 1) * TQ],
                                        start=(kt == 0), stop=(kt == KT - 1))

                            for kt in range(KT):
                                if first and hp == 0 and kt in (4, 8, 12):
                                    drain_label(("k0", kt // 4))
                                if first:
                                    proj_v(kt, hp)
                                s_t(kt)
                                if kt >= 1:
                                    pv(kt - 1)
                                step_filler(4 if first else 2)
                            pv(KT - 1)
                            if pidx == QC - 1 and hp == HP - 1:
                                for wi in range(10):
                                    wps = ps_tile("warm")
                                    nc.tensor.matmul(
                                        wps[:, 0:512],
                                        xT_sb[:, 0, 0:128],
                                        xT_sb[:, 0, 0:TQ],
                                        start=True, stop=True)

                            # stage P@V to SBUF (psum recycles), gather the
                            # denominator rows, then normalize this hp
                            for h in (0, 1):
                                # previous chunk's norm of this tile must
                                # not be overtaken
                                drain_label(("n", hp, h))
                                nc.vector.tensor_copy(pav_sb[(hp, h)][:],
                                                      pavs[h][:])
                                nc.gpsimd.dma_start(
                                    den_sb[pb][2 * hp + h:2 * hp + h + 1,
                                               :],
                                    pav_sb[(hp, h)][64:65, :])
                            recip_hp(pb)
                            for h in (0, 1):
                                queue.append((("n", hp, h),
                                              g_norm(pb, hp, h)))

                            if hp == HP - 2 and pidx < QC - 1:
                                # next chunk's first Q projection, early
                                # enough that the next chunk's first S_T
                                # never waits on it
                                queue.append((("q", 0), g_proj(
                                    wq_sb, 0, qT_sb, 0, (qc + 1) * TQ,
                                    ((qc + 1) % 2) * TQ)))

                    # ---- tail: last chunk's output projection (norms ran
                    # as filler already), its RS halves, and the two
                    # remaining bias+store passes
                    drain_filler()
                    for co in range(CO):
                        run_inline(g_out_proj(1, co, QC - 1))
                        if co == 4:
                            run_inline(gen_rs(QC - 1, 0))
                    run_inline(gen_rs(QC - 1, 1))
                    # chunk-2 bias+store was ready since mid-chunk 3; run
                    # it inside the RS wait window (Sync queue idle there)
                    # instead of contending with the outproj rs_in DMAs
                    final(QC - 2)
                    final(QC - 1)

    nc.compile()
    return nc


def _prep_inputs(hidden_states, Wq, Wk, Wv, Wo, bo):
    bf = ml_dtypes.bfloat16
    x = np.asarray(hidden_states, np.float32)
    Wq = np.asarray(Wq, np.float32)
    Wk = np.asarray(Wk, np.float32)
    Wv = np.asarray(Wv, np.float32)
    Wo = np.asarray(Wo, np.float32)
    bo_c = np.asarray(bo, np.float32).reshape(C, 1)
    sel = np.zeros((HL, HP, 2, 64), np.float32)
    for hp in range(HP):
        sel[2 * hp, hp, 0, :] = 1.0
        sel[2 * hp + 1, hp, 1, :] = 1.0
    sel = sel.reshape(HL, HP * 128).astype(bf)
    in_maps = []
    for r in range(N_CORES):
        b, h2 = r // 2, r % 2
        sl = slice(h2 * CL, (h2 + 1) * CL)
        xTr = np.ascontiguousarray(x[b].T).astype(bf)
        in_maps.append({
            "xT": xTr,
            "wvT": np.ascontiguousarray(Wv[sl, :].T).astype(bf),
            "wqT": np.ascontiguousarray(Wq[sl, :].T).astype(bf),
            "wkT": np.ascontiguousarray(Wk[sl, :].T).astype(bf),
            "woT": np.ascontiguousarray(Wo[:, sl].T).astype(bf),
            "bo": bo_c,
            "sel": sel,
        })
    return in_maps


def kernel(hidden_states, Wq, Wk, Wv, Wo, bo):
    global LAST_EXEC_TIME_NS
    _install_ntff_hook()
    Bx, Tx, Cx = hidden_states.shape
    assert (Bx, Tx, Cx) == (B, T, C)
    if "nc" not in _BUILD_CACHE:
        _BUILD_CACHE["nc"] = build()
    nc = _BUILD_CACHE["nc"]
    in_maps = _prep_inputs(hidden_states, Wq, Wk, Wv, Wo, bo)
    res = run_bass_kernel_spmd(nc, in_maps, core_ids=list(range(N_CORES)))
    LAST_EXEC_TIME_NS = res.exec_time_ns
    outf = np.empty((B, T, C), np.float32)
    for r in range(N_CORES):
        b, h2 = r // 2, r % 2
        yT = res.results[r]["out"]          # [C, QC*256]
        for qc in range(QC):
            t0 = qc * TQ + h2 * 256
            outf[b, t0:t0 + 256, :] = yT[:, qc * 256:(qc + 1) * 256].T
    return outf
